# revision 21
# baseline (speedup 1.0000x reference)
"""Trainium2 Bass kernel for nn_AttentionLayer (sparse windowed attention).

Reference computation (B=32, Tq=Tk=1024, C=E=256):
    x      = query @ W_in.T + b_in
    scores = x @ keys                      # [B, Tq, Tk]
    scores = where(pad_mask | ~window, -1e30, scores)   # window keeps cols [la, la+w)
    attn   = softmax(scores, -1)           # exact zeros outside the window
    out    = (attn @ values) * sqrt(Tk)
    out    = out @ W_out.T + b_out
    out    = (out + query) * sqrt(0.5)
    return out, attn

Only `window_size` (=3) columns of the score matrix survive the window mask, and
x is used *only* for scores.  Both projections therefore fold into tiny
per-batch matrices computed on the host in float64:
    kq[b] = W_in.T @ keys[b,:,lo:hi]                       # [C, nW]
    vw[b] = values[b,lo:hi,:] @ W_out.T * sqrt(Tk)*sqrt(.5)  # [nW, C]
so on-device work per batch is just
    scores = q @ kq ; attn = softmax(scores) ; out2 = attn @ vw
query is pre-scaled by sqrt(0.5) on the host (kq compensated by 1/sqrt(0.5));
the residual (out2 + q*sqrt(.5)) and the full attn matrix are assembled during
the host-side gather: all columns outside the window are exact 0.0 in the
reference (exp(-1e30 - max) underflows), so zeros + the device-computed window
columns reproduce it exactly.

Device pipeline per 512-row chunk (per-core, 4 batches x 2 chunks):
  DMA qT chunk -> PE: scores_T[w,t] = kq.T @ qT (fp32, kq stationary)
  -> PE transpose-back to [t,w] -> DVE/ACT softmax (batched, 0-stride
  broadcasts) -> PE transpose attn to [w,t] (f32r) -> PE out2 = attnT.T @ vw
  (f32r, full-rate) -> copy -> DMA out.  A short identity-transpose burst at
  kernel start warms the PE HAM clock-gate during the first DMA wait.

Sharding: data-parallel over batch, B=32 -> 4 batches on each of 8 cores.
"""
import math
import os
import sys

for _p in ("/opt/trn_rl_repo", "/root/.axon_site/_ro/trn_rl_repo"):
    if os.path.isdir(_p) and _p not in sys.path:
        sys.path.append(_p)

import numpy as np
import concourse.bacc as bacc
import concourse.bass as bass
import concourse.tile as tile
import concourse.mybir as mybir
from concourse.bass_utils import run_bass_kernel_spmd

F32 = mybir.dt.float32
F32R = mybir.dt.float32r
N_CORES = 8
SQ05 = math.sqrt(0.5)

_cache = {}


def _build(B_loc, T, C, nW, use_sbias, nomax):
    """Per-core kernel: [B_loc, T, C] queries, nW-column windowed attention."""
    assert C % 128 == 0 and 1 <= nW <= 128
    TC = next(tc for tc in (512, 256, 128) if T % tc == 0)
    n_sub = TC // 128             # 128-row sub-tiles per chunk
    n_chunks = T // TC
    nct = C // 128                # contraction tiles for scores

    nc = bacc.Bacc(None, target_bir_lowering=False, debug=False,
                   num_devices=N_CORES)
    qt_d = nc.dram_tensor("qsT", [B_loc, C, T], F32, kind="ExternalInput")
    id_d = nc.dram_tensor("ident", [128, 128], F32, kind="ExternalInput")
    kq_d = nc.dram_tensor("kq", [B_loc, C, nW], F32, kind="ExternalInput")
    vw_d = nc.dram_tensor("vw", [B_loc, nW, C], F32, kind="ExternalInput")
    if use_sbias:
        sb_d = nc.dram_tensor("sbias", [B_loc, nW], F32, kind="ExternalInput")
    out_d = nc.dram_tensor("out", [B_loc, T, C], F32, kind="ExternalOutput")
    attn_d = nc.dram_tensor("attnw", [B_loc, T, nW], F32, kind="ExternalOutput")

    with tile.TileContext(nc) as tc:
        with (
            tc.tile_pool(name="consts", bufs=1) as consts,
            tc.tile_pool(name="qtp", bufs=4) as qtp,
            tc.tile_pool(name="sct", bufs=3) as sctp,
            tc.tile_pool(name="smx", bufs=4) as smx,
            tc.tile_pool(name="atp", bufs=4) as atp,
            tc.tile_pool(name="fin", bufs=6) as finp,
            tc.tile_pool(name="pst", bufs=2, space="PSUM") as pst,
            tc.tile_pool(name="pss", bufs=2, space="PSUM") as pss,
            tc.tile_pool(name="psa", bufs=1, space="PSUM") as psa,
            tc.tile_pool(name="pso", bufs=3, space="PSUM") as pso,
        ):
            # small consts on the Scalar HWDGE queue so they land without
            # queueing behind the 512KB qT transfer on Sync
            ident = consts.tile([128, 128], F32)
            nc.scalar.dma_start(out=ident, in_=id_d.ap())
            kq_sb = consts.tile([128, B_loc, nct, nW], F32)
            nc.scalar.dma_start(
                out=kq_sb,
                in_=kq_d.ap().rearrange("b (ct p) w -> p b ct w", p=128))
            qt_first = qtp.tile([128, nct, TC], F32, tag="qt")
            nc.sync.dma_start(
                out=qt_first,
                in_=qt_d.ap()[0, :, 0:TC].rearrange("(ct p) t -> p ct t", p=128))
            vw_sb = consts.tile([nW, B_loc, C], F32)
            nc.scalar.dma_start(out=vw_sb, in_=vw_d.ap().rearrange("b w c -> w b c"))
            vw_r = consts.tile([nW, B_loc, C], F32R)
            nc.vector.tensor_copy(vw_r, vw_sb)
            if use_sbias:
                sb_sb = consts.tile([128, B_loc, nW], F32)
                nc.gpsimd.dma_start(
                    out=sb_sb,
                    in_=bass.AP(tensor=sb_d.ap().tensor, offset=0,
                                ap=[[0, 128], [nW, B_loc], [1, nW]]))

            # warm the PE clock-gate (HAM) during the initial DMA wait
            warm_ps = pst.tile([128, 128], F32, tag="scT")
            for _ in range(8):
                nc.tensor.transpose(warm_ps, ident, ident)

            for b in range(B_loc):
                attn_bt = smx.tile([128, n_chunks, n_sub, nW], F32, tag="attn")
                for ch in range(n_chunks):
                    t0 = ch * TC

                    # qT loaded directly (host supplies the transposed copy)
                    if b == 0 and ch == 0:
                        qt_sb = qt_first
                    else:
                        qt_sb = qtp.tile([128, nct, TC], F32, tag="qt")
                        nc.sync.dma_start(
                            out=qt_sb,
                            in_=qt_d.ap()[b, :, t0:t0 + TC]
                                .rearrange("(ct p) t -> p ct t", p=128))

                    # scores_T[w, t] = kq[b].T @ qT  (kq stationary: tiny LDW)
                    scT_ps = pst.tile([nW, TC], F32, tag="scT")
                    for ct in range(nct):
                        nc.tensor.matmul(
                            scT_ps, kq_sb[:, b, ct, :], qt_sb[:, ct, :],
                            start=(ct == 0), stop=(ct == nct - 1))
                    scT_sb = sctp.tile([nW, TC], F32, tag="scT_sb")
                    nc.any.tensor_copy(scT_sb, scT_ps)

                    # transpose back to [t, w] for the row-parallel softmax
                    sc_ps = pss.tile([128, n_sub, nW], F32, tag="sc_ps")
                    for s in range(n_sub):
                        nc.tensor.transpose(
                            sc_ps[:, s, :],
                            scT_sb[:, s * 128:(s + 1) * 128],
                            ident[0:nW, 0:nW])
                    if use_sbias:
                        for s in range(n_sub):
                            nc.vector.tensor_tensor(
                                sc_ps[:, s, :], sc_ps[:, s, :], sb_sb[:, b, :],
                                op=mybir.AluOpType.add)

                    # softmax over the nW window columns (batched over n_sub;
                    # max-subtraction skipped when host proves exp can't
                    # overflow for this data)
                    if nomax:
                        ex = smx.tile([128, n_sub, nW], F32, tag="ex")
                        nc.scalar.activation(ex, sc_ps,
                                             mybir.ActivationFunctionType.Exp)
                    else:
                        negmax = smx.tile([128, n_sub], F32, tag="negmax")
                        nc.vector.tensor_reduce(
                            negmax, sc_ps, axis=mybir.AxisListType.X,
                            op=mybir.AluOpType.max, negate=True)
                        negmax_bc = negmax[:, :, None].broadcast_to([128, n_sub, nW])
                        sh = smx.tile([128, n_sub, nW], F32, tag="sh")
                        nc.vector.tensor_tensor(sh, sc_ps, negmax_bc,
                                                op=mybir.AluOpType.add)
                        ex = smx.tile([128, n_sub, nW], F32, tag="ex")
                        nc.scalar.activation(ex, sh,
                                             mybir.ActivationFunctionType.Exp)
                    denom = smx.tile([128, n_sub], F32, tag="denom")
                    nc.vector.tensor_reduce(
                        denom, ex, axis=mybir.AxisListType.X,
                        op=mybir.AluOpType.add)
                    recip = smx.tile([128, n_sub], F32, tag="recip")
                    nc.vector.reciprocal(recip, denom)
                    attn_sb = attn_bt[:, ch, :, :]
                    nc.vector.tensor_tensor(attn_sb, ex,
                                            recip[:, :, None].broadcast_to([128, n_sub, nW]),
                                            op=mybir.AluOpType.mult)
                    if ch == n_chunks - 1:
                        nc.scalar.dma_start(
                            out=attn_d.ap()[b, :, :]
                                .rearrange("(c s p) w -> p c s w", p=128, s=n_sub),
                            in_=attn_bt)

                    # attn transposed to [w, t] for the output contraction
                    at_ps = psa.tile([nW, TC], F32, tag="at_ps")
                    for s in range(n_sub):
                        nc.tensor.transpose(
                            at_ps[:, s * 128:(s + 1) * 128],
                            attn_sb[:, s, :], ident)
                    at_r = atp.tile([nW, TC], F32R, tag="at_r")
                    nc.any.tensor_copy(at_r, at_ps)

                    # out2[t, c] = attn @ vw[b]  (residual added host-side)
                    fin = finp.tile([128, n_sub, C], F32, tag="fin")
                    for pair in range(n_sub // 2):
                        o2 = pso.tile([128, 2, C], F32, tag="o2")
                        for j in range(2):
                            s = pair * 2 + j
                            nc.tensor.matmul(
                                o2[:, j, :],
                                at_r[:, s * 128:(s + 1) * 128],
                                vw_r[:, b, :], start=True, stop=True)
                        nc.any.tensor_copy(fin[:, pair * 2:pair * 2 + 2, :], o2)
                    nc.scalar.dma_start(
                        out=out_d.ap()[b, t0:t0 + TC, :]
                            .rearrange("(s p) c -> p s c", p=128),
                        in_=fin)
    nc.compile()
    return nc


def kernel(query, keys, values, mask, W_in, b_in, W_out, b_out,
           last_attended, window_size):
    query = np.asarray(query, dtype=np.float32)
    keys = np.asarray(keys, dtype=np.float32)
    values = np.asarray(values, dtype=np.float32)
    mask = np.asarray(mask)
    W_in = np.asarray(W_in, dtype=np.float32)
    b_in = np.asarray(b_in, dtype=np.float32)
    W_out = np.asarray(W_out, dtype=np.float32)
    b_out = np.asarray(b_out, dtype=np.float32)
    la = int(last_attended)
    win = int(window_size)

    B, Tq, C = query.shape
    _, E, Tk = keys.shape
    lo, hi = max(0, la), min(Tk, la + win)
    assert hi > lo, "window has no valid column"
    nW = hi - lo
    assert B % N_CORES == 0
    B_loc = B // N_CORES

    # ---- host-side folding (float64) ----
    k_win = keys[:, :, lo:hi].astype(np.float64)          # [B, E, nW]
    v_win = values[:, lo:hi, :].astype(np.float64)        # [B, nW, E]
    mask_win = mask[:, lo:hi].astype(bool)                # [B, nW]
    out_scale = Tk * math.sqrt(1.0 / Tk)                  # = sqrt(Tk)

    # scores = (q @ W_in.T + b_in) @ k_win ; q fed pre-scaled by sqrt(.5)
    kq = np.einsum("ec,bew->bcw", W_in.astype(np.float64), k_win) / SQ05
    kq = np.ascontiguousarray(kq, dtype=np.float32)       # [B, C, nW]
    sbias = np.einsum("e,bew->bw", b_in.astype(np.float64), k_win)
    sbias = np.where(mask_win, -1e30, sbias).astype(np.float32)
    use_sbias = bool(np.any(sbias != 0.0))

    vw = np.einsum("bwe,ce->bwc", v_win, W_out.astype(np.float64))
    vw = np.ascontiguousarray(vw * (out_scale * SQ05), dtype=np.float32)
    qs = (query * np.float32(SQ05)).astype(np.float32)
    qsT = np.ascontiguousarray(qs.transpose(0, 2, 1))

    ident_np = np.eye(128, dtype=np.float32)
    key = (B_loc, Tq, C, nW, use_sbias, False)
    if key not in _cache:
        _cache[key] = _build(*key)
    nc = _cache[key]

    in_maps = []
    for c in range(N_CORES):
        s = slice(c * B_loc, (c + 1) * B_loc)
        m = {"qsT": qsT[s], "kq": kq[s], "vw": vw[s], "ident": ident_np}
        if use_sbias:
            m["sbias"] = sbias[s]
        in_maps.append(m)

    kw = {}
    if os.environ.get("BASS_KERNEL_TRACE"):
        try:
            from antenv.axon_hooks import get_axon_ntff_profile_hook  # noqa: F401
            import tempfile
            base = os.environ.get("BASS_KERNEL_TRACE_DIR") or tempfile.gettempdir()
            os.makedirs(base, exist_ok=True)
            kw = {"trace": True, "tmpdir": tempfile.mkdtemp(dir=base)}
        except ImportError:
            pass
    res = run_bass_kernel_spmd(nc, in_maps, list(range(N_CORES)), **kw)
    kernel.last_results = res

    out = np.concatenate([r["out"] for r in res.results], axis=0)
    out += qs
    if np.any(b_out != 0.0):
        out = out + (b_out * SQ05).astype(np.float32)
    attn = np.zeros((B, Tq, Tk), dtype=np.float32)
    attn[:, :, lo:hi] = np.concatenate([r["attnw"] for r in res.results], axis=0)
    return out, attn


# revision 22
# speedup vs baseline: 1.0581x; 1.0581x over previous
"""Trainium2 Bass kernel for nn_AttentionLayer (sparse windowed attention).

Reference computation (B=32, Tq=Tk=1024, C=E=256):
    x      = query @ W_in.T + b_in
    scores = x @ keys                      # [B, Tq, Tk]
    scores = where(pad_mask | ~window, -1e30, scores)   # window keeps cols [la, la+w)
    attn   = softmax(scores, -1)           # exact zeros outside the window
    out    = (attn @ values) * sqrt(Tk)
    out    = out @ W_out.T + b_out
    out    = (out + query) * sqrt(0.5)
    return out, attn

Only `window_size` (=3) columns of the score matrix survive the window mask, and
x is used *only* for scores.  Both projections therefore fold into tiny
per-batch matrices computed on the host in float64:
    kq[b] = W_in.T @ keys[b,:,lo:hi]                       # [C, nW]
    vw[b] = values[b,lo:hi,:] @ W_out.T * sqrt(Tk)*sqrt(.5)  # [nW, C]
so on-device work per batch is just
    scores = q @ kq ; attn = softmax(scores) ; out2 = attn @ vw
query is pre-scaled by sqrt(0.5) on the host (kq compensated by 1/sqrt(0.5));
the residual (out2 + q*sqrt(.5)) and the full attn matrix are assembled during
the host-side gather: all columns outside the window are exact 0.0 in the
reference (exp(-1e30 - max) underflows), so zeros + the device-computed window
columns reproduce it exactly.

Device pipeline per 512-row chunk (per-core, 4 batches x 2 chunks):
  DMA qT chunk -> PE: scores_T[w,t] = kq.T @ qT (fp32, kq stationary)
  -> PE transpose-back to [t,w] -> DVE/ACT softmax (batched, 0-stride
  broadcasts) -> PE transpose attn to [w,t] (f32r) -> PE out2 = attnT.T @ vw
  (f32r, full-rate) -> copy -> DMA out.  A short identity-transpose burst at
  kernel start warms the PE HAM clock-gate during the first DMA wait.

Sharding: data-parallel over batch, B=32 -> 4 batches on each of 8 cores.
"""
import math
import os
import sys

for _p in ("/opt/trn_rl_repo", "/root/.axon_site/_ro/trn_rl_repo"):
    if os.path.isdir(_p) and _p not in sys.path:
        sys.path.append(_p)

import numpy as np
import concourse.bacc as bacc
import concourse.bass as bass
import concourse.tile as tile
import concourse.mybir as mybir
from concourse.bass_utils import run_bass_kernel_spmd

F32 = mybir.dt.float32
F32R = mybir.dt.float32r
N_CORES = 8
SQ05 = math.sqrt(0.5)

_cache = {}


def _build(B_loc, T, C, nW, use_sbias, nomax):
    """Per-core kernel: [B_loc, T, C] queries, nW-column windowed attention."""
    assert C % 128 == 0 and 1 <= nW <= 128
    TC = next(tc for tc in (512, 256, 128) if T % tc == 0)
    n_sub = TC // 128             # 128-row sub-tiles per chunk
    n_chunks = T // TC
    nct = C // 128                # contraction tiles for scores

    nc = bacc.Bacc(None, target_bir_lowering=False, debug=False,
                   num_devices=N_CORES)
    qt_d = nc.dram_tensor("qsT", [B_loc, C, T], F32, kind="ExternalInput")
    id_d = nc.dram_tensor("ident", [128, 128], F32, kind="ExternalInput")
    kq_d = nc.dram_tensor("kq", [B_loc, C, nW], F32, kind="ExternalInput")
    vw_d = nc.dram_tensor("vw", [B_loc, nW, C], F32, kind="ExternalInput")
    if use_sbias:
        sb_d = nc.dram_tensor("sbias", [B_loc, nW], F32, kind="ExternalInput")
    out_d = nc.dram_tensor("out", [B_loc, T, C], F32, kind="ExternalOutput")
    attn_d = nc.dram_tensor("attnw", [B_loc, T, nW], F32, kind="ExternalOutput")

    with tile.TileContext(nc) as tc:
        with (
            tc.tile_pool(name="consts", bufs=1) as consts,
            tc.tile_pool(name="qtp", bufs=4) as qtp,
            tc.tile_pool(name="sct", bufs=3) as sctp,
            tc.tile_pool(name="smx", bufs=4) as smx,
            tc.tile_pool(name="atp", bufs=4) as atp,
            tc.tile_pool(name="fin", bufs=6) as finp,
            tc.tile_pool(name="pst", bufs=2, space="PSUM") as pst,
            tc.tile_pool(name="pss", bufs=2, space="PSUM") as pss,
            tc.tile_pool(name="psa", bufs=1, space="PSUM") as psa,
            tc.tile_pool(name="pso", bufs=3, space="PSUM") as pso,
        ):
            qt_first = qtp.tile([128, nct, TC], F32, tag="qt")
            nc.sync.dma_start(
                out=qt_first,
                in_=qt_d.ap()[0, :, 0:TC].rearrange("(ct p) t -> p ct t", p=128))
            ident = consts.tile([128, 128], F32)
            nc.sync.dma_start(out=ident, in_=id_d.ap())
            kq_sb = consts.tile([128, B_loc, nct, nW], F32)
            nc.sync.dma_start(
                out=kq_sb,
                in_=kq_d.ap().rearrange("b (ct p) w -> p b ct w", p=128))
            vw_sb = consts.tile([nW, B_loc, C], F32)
            nc.sync.dma_start(out=vw_sb, in_=vw_d.ap().rearrange("b w c -> w b c"))
            vw_r = consts.tile([nW, B_loc, C], F32R)
            nc.vector.tensor_copy(vw_r, vw_sb)
            if use_sbias:
                sb_sb = consts.tile([128, B_loc, nW], F32)
                nc.gpsimd.dma_start(
                    out=sb_sb,
                    in_=bass.AP(tensor=sb_d.ap().tensor, offset=0,
                                ap=[[0, 128], [nW, B_loc], [1, nW]]))

            # warm the PE clock-gate (HAM) during the initial DMA wait
            warm_ps = pst.tile([128, 128], F32, tag="scT")
            for _ in range(8):
                nc.tensor.transpose(warm_ps, ident, ident)

            for b in range(B_loc):
                attn_bt = smx.tile([128, n_chunks, n_sub, nW], F32, tag="attn")
                for ch in range(n_chunks):
                    t0 = ch * TC

                    # qT loaded directly (host supplies the transposed copy)
                    if b == 0 and ch == 0:
                        qt_sb = qt_first
                    else:
                        qt_sb = qtp.tile([128, nct, TC], F32, tag="qt")
                        nc.sync.dma_start(
                            out=qt_sb,
                            in_=qt_d.ap()[b, :, t0:t0 + TC]
                                .rearrange("(ct p) t -> p ct t", p=128))

                    # scores_T[w, t] = kq[b].T @ qT  (kq stationary: tiny LDW)
                    scT_ps = pst.tile([nW, TC], F32, tag="scT")
                    for ct in range(nct):
                        nc.tensor.matmul(
                            scT_ps, kq_sb[:, b, ct, :], qt_sb[:, ct, :],
                            start=(ct == 0), stop=(ct == nct - 1))
                    scT_sb = sctp.tile([nW, TC], F32, tag="scT_sb")
                    nc.any.tensor_copy(scT_sb, scT_ps)

                    # transpose back to [t, w] for the row-parallel softmax
                    sc_ps = pss.tile([128, n_sub, nW], F32, tag="sc_ps")
                    for s in range(n_sub):
                        nc.tensor.transpose(
                            sc_ps[:, s, :],
                            scT_sb[:, s * 128:(s + 1) * 128],
                            ident[0:nW, 0:nW])
                    if use_sbias:
                        for s in range(n_sub):
                            nc.vector.tensor_tensor(
                                sc_ps[:, s, :], sc_ps[:, s, :], sb_sb[:, b, :],
                                op=mybir.AluOpType.add)

                    # softmax over the nW window columns (batched over n_sub;
                    # max-subtraction skipped when host proves exp can't
                    # overflow for this data)
                    if nomax:
                        ex = smx.tile([128, n_sub, nW], F32, tag="ex")
                        nc.scalar.activation(ex, sc_ps,
                                             mybir.ActivationFunctionType.Exp)
                    else:
                        negmax = smx.tile([128, n_sub], F32, tag="negmax")
                        nc.vector.tensor_reduce(
                            negmax, sc_ps, axis=mybir.AxisListType.X,
                            op=mybir.AluOpType.max, negate=True)
                        negmax_bc = negmax[:, :, None].broadcast_to([128, n_sub, nW])
                        sh = smx.tile([128, n_sub, nW], F32, tag="sh")
                        nc.vector.tensor_tensor(sh, sc_ps, negmax_bc,
                                                op=mybir.AluOpType.add)
                        ex = smx.tile([128, n_sub, nW], F32, tag="ex")
                        nc.scalar.activation(ex, sh,
                                             mybir.ActivationFunctionType.Exp)
                    denom = smx.tile([128, n_sub], F32, tag="denom")
                    nc.vector.tensor_reduce(
                        denom, ex, axis=mybir.AxisListType.X,
                        op=mybir.AluOpType.add)
                    recip = smx.tile([128, n_sub], F32, tag="recip")
                    nc.vector.reciprocal(recip, denom)
                    attn_sb = attn_bt[:, ch, :, :]
                    nc.vector.tensor_tensor(attn_sb, ex,
                                            recip[:, :, None].broadcast_to([128, n_sub, nW]),
                                            op=mybir.AluOpType.mult)
                    if ch == n_chunks - 1:
                        nc.scalar.dma_start(
                            out=attn_d.ap()[b, :, :]
                                .rearrange("(c s p) w -> p c s w", p=128, s=n_sub),
                            in_=attn_bt)

                    # attn transposed to [w, t] for the output contraction
                    at_ps = psa.tile([nW, TC], F32, tag="at_ps")
                    for s in range(n_sub):
                        nc.tensor.transpose(
                            at_ps[:, s * 128:(s + 1) * 128],
                            attn_sb[:, s, :], ident)
                    at_r = atp.tile([nW, TC], F32R, tag="at_r")
                    nc.any.tensor_copy(at_r, at_ps)

                    # out2[t, c] = attn @ vw[b]  (residual added host-side)
                    fin = finp.tile([128, n_sub, C], F32, tag="fin")
                    for pair in range(n_sub // 2):
                        o2 = pso.tile([128, 2, C], F32, tag="o2")
                        for j in range(2):
                            s = pair * 2 + j
                            nc.tensor.matmul(
                                o2[:, j, :],
                                at_r[:, s * 128:(s + 1) * 128],
                                vw_r[:, b, :], start=True, stop=True)
                        nc.any.tensor_copy(fin[:, pair * 2:pair * 2 + 2, :], o2)
                    nc.scalar.dma_start(
                        out=out_d.ap()[b, t0:t0 + TC, :]
                            .rearrange("(s p) c -> p s c", p=128),
                        in_=fin)
    nc.compile()
    return nc


def kernel(query, keys, values, mask, W_in, b_in, W_out, b_out,
           last_attended, window_size):
    query = np.asarray(query, dtype=np.float32)
    keys = np.asarray(keys, dtype=np.float32)
    values = np.asarray(values, dtype=np.float32)
    mask = np.asarray(mask)
    W_in = np.asarray(W_in, dtype=np.float32)
    b_in = np.asarray(b_in, dtype=np.float32)
    W_out = np.asarray(W_out, dtype=np.float32)
    b_out = np.asarray(b_out, dtype=np.float32)
    la = int(last_attended)
    win = int(window_size)

    B, Tq, C = query.shape
    _, E, Tk = keys.shape
    lo, hi = max(0, la), min(Tk, la + win)
    assert hi > lo, "window has no valid column"
    nW = hi - lo
    assert B % N_CORES == 0
    B_loc = B // N_CORES

    # ---- host-side folding (float64) ----
    k_win = keys[:, :, lo:hi].astype(np.float64)          # [B, E, nW]
    v_win = values[:, lo:hi, :].astype(np.float64)        # [B, nW, E]
    mask_win = mask[:, lo:hi].astype(bool)                # [B, nW]
    out_scale = Tk * math.sqrt(1.0 / Tk)                  # = sqrt(Tk)

    # scores = (q @ W_in.T + b_in) @ k_win ; q fed pre-scaled by sqrt(.5)
    kq = np.einsum("ec,bew->bcw", W_in.astype(np.float64), k_win) / SQ05
    kq = np.ascontiguousarray(kq, dtype=np.float32)       # [B, C, nW]
    sbias = np.einsum("e,bew->bw", b_in.astype(np.float64), k_win)
    sbias = np.where(mask_win, -1e30, sbias).astype(np.float32)
    use_sbias = bool(np.any(sbias != 0.0))

    vw = np.einsum("bwe,ce->bwc", v_win, W_out.astype(np.float64))
    vw = np.ascontiguousarray(vw * (out_scale * SQ05), dtype=np.float32)
    qs = (query * np.float32(SQ05)).astype(np.float32)
    qsT = np.ascontiguousarray(qs.transpose(0, 2, 1))

    ident_np = np.eye(128, dtype=np.float32)
    key = (B_loc, Tq, C, nW, use_sbias, False)
    if key not in _cache:
        _cache[key] = _build(*key)
    nc = _cache[key]

    in_maps = []
    for c in range(N_CORES):
        s = slice(c * B_loc, (c + 1) * B_loc)
        m = {"qsT": qsT[s], "kq": kq[s], "vw": vw[s], "ident": ident_np}
        if use_sbias:
            m["sbias"] = sbias[s]
        in_maps.append(m)

    kw = {}
    if os.environ.get("BASS_KERNEL_TRACE"):
        try:
            from antenv.axon_hooks import get_axon_ntff_profile_hook  # noqa: F401
            import tempfile
            base = os.environ.get("BASS_KERNEL_TRACE_DIR") or tempfile.gettempdir()
            os.makedirs(base, exist_ok=True)
            kw = {"trace": True, "tmpdir": tempfile.mkdtemp(dir=base)}
        except ImportError:
            pass
    res = run_bass_kernel_spmd(nc, in_maps, list(range(N_CORES)), **kw)
    kernel.last_results = res

    out = np.concatenate([r["out"] for r in res.results], axis=0)
    out += qs
    if np.any(b_out != 0.0):
        out = out + (b_out * SQ05).astype(np.float32)
    attn = np.zeros((B, Tq, Tk), dtype=np.float32)
    attn[:, :, lo:hi] = np.concatenate([r["attnw"] for r in res.results], axis=0)
    return out, attn


# revision 23
# speedup vs baseline: 1.0849x; 1.0254x over previous
"""Trainium2 Bass kernel for nn_AttentionLayer (sparse windowed attention).

Reference computation (B=32, Tq=Tk=1024, C=E=256):
    x      = query @ W_in.T + b_in
    scores = x @ keys                      # [B, Tq, Tk]
    scores = where(pad_mask | ~window, -1e30, scores)   # window keeps cols [la, la+w)
    attn   = softmax(scores, -1)           # exact zeros outside the window
    out    = (attn @ values) * sqrt(Tk)
    out    = out @ W_out.T + b_out
    out    = (out + query) * sqrt(0.5)
    return out, attn

Only `window_size` (=3) columns of the score matrix survive the window mask, and
x is used *only* for scores.  Both projections therefore fold into tiny
per-batch matrices computed on the host in float64:
    kq[b] = W_in.T @ keys[b,:,lo:hi]                       # [C, nW]
    vw[b] = values[b,lo:hi,:] @ W_out.T * sqrt(Tk)*sqrt(.5)  # [nW, C]
so on-device work per batch is just
    scores = q @ kq ; attn = softmax(scores) ; out2 = attn @ vw
query is pre-scaled by sqrt(0.5) on the host (kq compensated by 1/sqrt(0.5));
the residual (out2 + q*sqrt(.5)) and the full attn matrix are assembled during
the host-side gather: all columns outside the window are exact 0.0 in the
reference (exp(-1e30 - max) underflows), so zeros + the device-computed window
columns reproduce it exactly.

Device pipeline per 512-row chunk (per-core, 4 batches x 2 chunks):
  DMA qT chunk -> PE: scores_T[w,t] = kq.T @ qT (fp32, kq stationary)
  -> PE transpose-back to [t,w] -> DVE/ACT softmax (batched, 0-stride
  broadcasts) -> PE transpose attn to [w,t] (f32r) -> PE out2 = attnT.T @ vw
  (f32r, full-rate) -> copy -> DMA out.  A short identity-transpose burst at
  kernel start warms the PE HAM clock-gate during the first DMA wait.

Sharding: data-parallel over batch, B=32 -> 4 batches on each of 8 cores.
"""
import math
import os
import sys

for _p in ("/opt/trn_rl_repo", "/root/.axon_site/_ro/trn_rl_repo"):
    if os.path.isdir(_p) and _p not in sys.path:
        sys.path.append(_p)

import numpy as np
import concourse.bacc as bacc
import concourse.bass as bass
import concourse.tile as tile
import concourse.mybir as mybir
from concourse.bass_utils import run_bass_kernel_spmd

F32 = mybir.dt.float32
F32R = mybir.dt.float32r
N_CORES = 8
SQ05 = math.sqrt(0.5)

_cache = {}


def _build(B_loc, T, C, nW, use_sbias, nomax):
    """Per-core kernel: [B_loc, T, C] queries, nW-column windowed attention."""
    assert C % 128 == 0 and 1 <= nW <= 128
    TC = next(tc for tc in (512, 256, 128) if T % tc == 0)
    n_sub = TC // 128             # 128-row sub-tiles per chunk
    n_chunks = T // TC
    nct = C // 128                # contraction tiles for scores

    nc = bacc.Bacc(None, target_bir_lowering=False, debug=False,
                   num_devices=N_CORES)
    qt_d = nc.dram_tensor("qsT", [B_loc, C, T], F32, kind="ExternalInput")
    id_d = nc.dram_tensor("ident", [128, 128], F32, kind="ExternalInput")
    kq_d = nc.dram_tensor("kq", [B_loc, C, nW], F32, kind="ExternalInput")
    vw_d = nc.dram_tensor("vw", [B_loc, nW, C], F32, kind="ExternalInput")
    if use_sbias:
        sb_d = nc.dram_tensor("sbias", [B_loc, nW], F32, kind="ExternalInput")
    out_d = nc.dram_tensor("out", [B_loc, T, C], F32, kind="ExternalOutput")
    attn_d = nc.dram_tensor("attnw", [B_loc, T, nW], F32, kind="ExternalOutput")

    with tile.TileContext(nc) as tc:
        with (
            tc.tile_pool(name="consts", bufs=1) as consts,
            tc.tile_pool(name="qtp", bufs=6) as qtp,
            tc.tile_pool(name="sct", bufs=3) as sctp,
            tc.tile_pool(name="smx", bufs=4) as smx,
            tc.tile_pool(name="atp", bufs=4) as atp,
            tc.tile_pool(name="fin", bufs=6) as finp,
            tc.tile_pool(name="pst", bufs=2, space="PSUM") as pst,
            tc.tile_pool(name="pss", bufs=2, space="PSUM") as pss,
            tc.tile_pool(name="psa", bufs=1, space="PSUM") as psa,
            tc.tile_pool(name="pso", bufs=3, space="PSUM") as pso,
        ):
            qt_first = qtp.tile([128, nct, TC], F32, tag="qt")
            nc.sync.dma_start(
                out=qt_first,
                in_=qt_d.ap()[0, :, 0:TC].rearrange("(ct p) t -> p ct t", p=128))
            kq_sb = consts.tile([128, B_loc, nct, nW], F32)
            nc.sync.dma_start(
                out=kq_sb,
                in_=kq_d.ap().rearrange("b (ct p) w -> p b ct w", p=128))
            ident = consts.tile([128, 128], F32)
            nc.sync.dma_start(out=ident, in_=id_d.ap())
            vw_sb = consts.tile([nW, B_loc, C], F32)
            nc.sync.dma_start(out=vw_sb, in_=vw_d.ap().rearrange("b w c -> w b c"))
            vw_r = consts.tile([nW, B_loc, C], F32R)
            nc.vector.tensor_copy(vw_r, vw_sb)
            if use_sbias:
                sb_sb = consts.tile([128, B_loc, nW], F32)
                nc.gpsimd.dma_start(
                    out=sb_sb,
                    in_=bass.AP(tensor=sb_d.ap().tensor, offset=0,
                                ap=[[0, 128], [nW, B_loc], [1, nW]]))

            # warm the PE clock-gate (HAM) during the initial DMA wait;
            # memset source so the warmup depends on no DMA
            warm_src = consts.tile([128, 128], F32)
            nc.vector.memset(warm_src, 1.0)
            warm_ps = pst.tile([128, 128], F32, tag="scT")
            for _ in range(10):
                nc.tensor.transpose(warm_ps, warm_src, warm_src)

            for b in range(B_loc):
                attn_bt = smx.tile([128, n_chunks, n_sub, nW], F32, tag="attn")
                for ch in range(n_chunks):
                    t0 = ch * TC

                    # qT loaded directly (host supplies the transposed copy)
                    if b == 0 and ch == 0:
                        qt_sb = qt_first
                    else:
                        qt_sb = qtp.tile([128, nct, TC], F32, tag="qt")
                        nc.sync.dma_start(
                            out=qt_sb,
                            in_=qt_d.ap()[b, :, t0:t0 + TC]
                                .rearrange("(ct p) t -> p ct t", p=128))

                    # scores_T[w, t] = kq[b].T @ qT  (kq stationary: tiny LDW)
                    scT_ps = pst.tile([nW, TC], F32, tag="scT")
                    for ct in range(nct):
                        nc.tensor.matmul(
                            scT_ps, kq_sb[:, b, ct, :], qt_sb[:, ct, :],
                            start=(ct == 0), stop=(ct == nct - 1))
                    scT_sb = sctp.tile([nW, TC], F32, tag="scT_sb")
                    nc.any.tensor_copy(scT_sb, scT_ps)

                    # transpose back to [t, w] for the row-parallel softmax
                    sc_ps = pss.tile([128, n_sub, nW], F32, tag="sc_ps")
                    for s in range(n_sub):
                        nc.tensor.transpose(
                            sc_ps[:, s, :],
                            scT_sb[:, s * 128:(s + 1) * 128],
                            ident[0:nW, 0:nW])
                    if use_sbias:
                        for s in range(n_sub):
                            nc.vector.tensor_tensor(
                                sc_ps[:, s, :], sc_ps[:, s, :], sb_sb[:, b, :],
                                op=mybir.AluOpType.add)

                    # softmax over the nW window columns (batched over n_sub;
                    # max-subtraction skipped when host proves exp can't
                    # overflow for this data)
                    if nomax:
                        ex = smx.tile([128, n_sub, nW], F32, tag="ex")
                        nc.scalar.activation(ex, sc_ps,
                                             mybir.ActivationFunctionType.Exp)
                    else:
                        negmax = smx.tile([128, n_sub], F32, tag="negmax")
                        nc.vector.tensor_reduce(
                            negmax, sc_ps, axis=mybir.AxisListType.X,
                            op=mybir.AluOpType.max, negate=True)
                        negmax_bc = negmax[:, :, None].broadcast_to([128, n_sub, nW])
                        sh = smx.tile([128, n_sub, nW], F32, tag="sh")
                        nc.vector.tensor_tensor(sh, sc_ps, negmax_bc,
                                                op=mybir.AluOpType.add)
                        ex = smx.tile([128, n_sub, nW], F32, tag="ex")
                        nc.scalar.activation(ex, sh,
                                             mybir.ActivationFunctionType.Exp)
                    denom = smx.tile([128, n_sub], F32, tag="denom")
                    nc.vector.tensor_reduce(
                        denom, ex, axis=mybir.AxisListType.X,
                        op=mybir.AluOpType.add)
                    recip = smx.tile([128, n_sub], F32, tag="recip")
                    nc.vector.reciprocal(recip, denom)
                    attn_sb = attn_bt[:, ch, :, :]
                    nc.vector.tensor_tensor(attn_sb, ex,
                                            recip[:, :, None].broadcast_to([128, n_sub, nW]),
                                            op=mybir.AluOpType.mult)
                    if ch == n_chunks - 1:
                        nc.scalar.dma_start(
                            out=attn_d.ap()[b, :, :]
                                .rearrange("(c s p) w -> p c s w", p=128, s=n_sub),
                            in_=attn_bt)

                    # attn transposed to [w, t] for the output contraction
                    at_ps = psa.tile([nW, TC], F32, tag="at_ps")
                    for s in range(n_sub):
                        nc.tensor.transpose(
                            at_ps[:, s * 128:(s + 1) * 128],
                            attn_sb[:, s, :], ident)
                    at_r = atp.tile([nW, TC], F32R, tag="at_r")
                    nc.any.tensor_copy(at_r, at_ps)

                    # out2[t, c] = attn @ vw[b]  (residual added host-side)
                    fin = finp.tile([128, n_sub, C], F32, tag="fin")
                    for pair in range(n_sub // 2):
                        o2 = pso.tile([128, 2, C], F32, tag="o2")
                        for j in range(2):
                            s = pair * 2 + j
                            nc.tensor.matmul(
                                o2[:, j, :],
                                at_r[:, s * 128:(s + 1) * 128],
                                vw_r[:, b, :], start=True, stop=True)
                        nc.any.tensor_copy(fin[:, pair * 2:pair * 2 + 2, :], o2)
                    nc.scalar.dma_start(
                        out=out_d.ap()[b, t0:t0 + TC, :]
                            .rearrange("(s p) c -> p s c", p=128),
                        in_=fin)
    nc.compile()
    return nc


def kernel(query, keys, values, mask, W_in, b_in, W_out, b_out,
           last_attended, window_size):
    query = np.asarray(query, dtype=np.float32)
    keys = np.asarray(keys, dtype=np.float32)
    values = np.asarray(values, dtype=np.float32)
    mask = np.asarray(mask)
    W_in = np.asarray(W_in, dtype=np.float32)
    b_in = np.asarray(b_in, dtype=np.float32)
    W_out = np.asarray(W_out, dtype=np.float32)
    b_out = np.asarray(b_out, dtype=np.float32)
    la = int(last_attended)
    win = int(window_size)

    B, Tq, C = query.shape
    _, E, Tk = keys.shape
    lo, hi = max(0, la), min(Tk, la + win)
    assert hi > lo, "window has no valid column"
    nW = hi - lo
    assert B % N_CORES == 0
    B_loc = B // N_CORES

    # ---- host-side folding (float64) ----
    k_win = keys[:, :, lo:hi].astype(np.float64)          # [B, E, nW]
    v_win = values[:, lo:hi, :].astype(np.float64)        # [B, nW, E]
    mask_win = mask[:, lo:hi].astype(bool)                # [B, nW]
    out_scale = Tk * math.sqrt(1.0 / Tk)                  # = sqrt(Tk)

    # scores = (q @ W_in.T + b_in) @ k_win ; q fed pre-scaled by sqrt(.5)
    kq = np.einsum("ec,bew->bcw", W_in.astype(np.float64), k_win) / SQ05
    kq = np.ascontiguousarray(kq, dtype=np.float32)       # [B, C, nW]
    sbias = np.einsum("e,bew->bw", b_in.astype(np.float64), k_win)
    sbias = np.where(mask_win, -1e30, sbias).astype(np.float32)
    use_sbias = bool(np.any(sbias != 0.0))

    vw = np.einsum("bwe,ce->bwc", v_win, W_out.astype(np.float64))
    vw = np.ascontiguousarray(vw * (out_scale * SQ05), dtype=np.float32)
    qs = (query * np.float32(SQ05)).astype(np.float32)
    qsT = np.ascontiguousarray(qs.transpose(0, 2, 1))

    ident_np = np.eye(128, dtype=np.float32)
    key = (B_loc, Tq, C, nW, use_sbias, False)
    if key not in _cache:
        _cache[key] = _build(*key)
    nc = _cache[key]

    in_maps = []
    for c in range(N_CORES):
        s = slice(c * B_loc, (c + 1) * B_loc)
        m = {"qsT": qsT[s], "kq": kq[s], "vw": vw[s], "ident": ident_np}
        if use_sbias:
            m["sbias"] = sbias[s]
        in_maps.append(m)

    kw = {}
    if os.environ.get("BASS_KERNEL_TRACE"):
        try:
            from antenv.axon_hooks import get_axon_ntff_profile_hook  # noqa: F401
            import tempfile
            base = os.environ.get("BASS_KERNEL_TRACE_DIR") or tempfile.gettempdir()
            os.makedirs(base, exist_ok=True)
            kw = {"trace": True, "tmpdir": tempfile.mkdtemp(dir=base)}
        except ImportError:
            pass
    res = run_bass_kernel_spmd(nc, in_maps, list(range(N_CORES)), **kw)
    kernel.last_results = res

    out = np.concatenate([r["out"] for r in res.results], axis=0)
    out += qs
    if np.any(b_out != 0.0):
        out = out + (b_out * SQ05).astype(np.float32)
    attn = np.zeros((B, Tq, Tk), dtype=np.float32)
    attn[:, :, lo:hi] = np.concatenate([r["attnw"] for r in res.results], axis=0)
    return out, attn


# revision 24
# speedup vs baseline: 1.1068x; 1.0201x over previous
"""Trainium2 Bass kernel for nn_AttentionLayer (sparse windowed attention).

Reference computation (B=32, Tq=Tk=1024, C=E=256):
    x      = query @ W_in.T + b_in
    scores = x @ keys                      # [B, Tq, Tk]
    scores = where(pad_mask | ~window, -1e30, scores)   # window keeps cols [la, la+w)
    attn   = softmax(scores, -1)           # exact zeros outside the window
    out    = (attn @ values) * sqrt(Tk)
    out    = out @ W_out.T + b_out
    out    = (out + query) * sqrt(0.5)
    return out, attn

Only `window_size` (=3) columns of the score matrix survive the window mask, and
x is used *only* for scores.  Both projections therefore fold into tiny
per-batch matrices computed on the host in float64:
    kq[b] = W_in.T @ keys[b,:,lo:hi]                       # [C, nW]
    vw[b] = values[b,lo:hi,:] @ W_out.T * sqrt(Tk)*sqrt(.5)  # [nW, C]
so on-device work per batch is just
    scores = q @ kq ; attn = softmax(scores) ; out2 = attn @ vw
query is pre-scaled by sqrt(0.5) on the host (kq compensated by 1/sqrt(0.5));
the residual (out2 + q*sqrt(.5)) and the full attn matrix are assembled during
the host-side gather: all columns outside the window are exact 0.0 in the
reference (exp(-1e30 - max) underflows), so zeros + the device-computed window
columns reproduce it exactly.

Device pipeline per 512-row chunk (per-core, 4 batches x 2 chunks):
  DMA qT chunk -> PE: scores_T[w,t] = kq.T @ qT (fp32, kq stationary)
  -> PE transpose-back to [t,w] -> DVE/ACT softmax (batched, 0-stride
  broadcasts) -> PE transpose attn to [w,t] (f32r) -> PE out2 = attnT.T @ vw
  (f32r, full-rate) -> copy -> DMA out.  A short identity-transpose burst at
  kernel start warms the PE HAM clock-gate during the first DMA wait.

Sharding: data-parallel over batch, B=32 -> 4 batches on each of 8 cores.
"""
import math
import os
import sys

for _p in ("/opt/trn_rl_repo", "/root/.axon_site/_ro/trn_rl_repo"):
    if os.path.isdir(_p) and _p not in sys.path:
        sys.path.append(_p)

import numpy as np
import concourse.bacc as bacc
import concourse.bass as bass
import concourse.tile as tile
import concourse.mybir as mybir
from concourse.bass_utils import run_bass_kernel_spmd

F32 = mybir.dt.float32
F32R = mybir.dt.float32r
N_CORES = 8
SQ05 = math.sqrt(0.5)

_cache = {}


def _build(B_loc, T, C, nW, use_sbias, nomax):
    """Per-core kernel: [B_loc, T, C] queries, nW-column windowed attention."""
    assert C % 128 == 0 and 1 <= nW <= 128
    TC = next(tc for tc in (512, 256, 128) if T % tc == 0)
    n_sub = TC // 128             # 128-row sub-tiles per chunk
    n_chunks = T // TC
    nct = C // 128                # contraction tiles for scores

    nc = bacc.Bacc(None, target_bir_lowering=False, debug=False,
                   num_devices=N_CORES)
    qt_d = nc.dram_tensor("qsT", [B_loc, C, T], F32, kind="ExternalInput")
    id_d = nc.dram_tensor("ident", [128, 128], F32, kind="ExternalInput")
    kq_d = nc.dram_tensor("kq", [B_loc, C, nW], F32, kind="ExternalInput")
    vw_d = nc.dram_tensor("vw", [B_loc, nW, C], F32, kind="ExternalInput")
    if use_sbias:
        sb_d = nc.dram_tensor("sbias", [B_loc, nW], F32, kind="ExternalInput")
    out_d = nc.dram_tensor("out", [B_loc, T, C], F32, kind="ExternalOutput")
    attn_d = nc.dram_tensor("attnw", [B_loc, T, nW], F32, kind="ExternalOutput")

    with tile.TileContext(nc) as tc:
        with (
            tc.tile_pool(name="consts", bufs=1) as consts,
            tc.tile_pool(name="qtp", bufs=6) as qtp,
            tc.tile_pool(name="sct", bufs=3) as sctp,
            tc.tile_pool(name="smx", bufs=4) as smx,
            tc.tile_pool(name="atp", bufs=4) as atp,
            tc.tile_pool(name="fin", bufs=6) as finp,
            tc.tile_pool(name="pst", bufs=2, space="PSUM") as pst,
            tc.tile_pool(name="pss", bufs=2, space="PSUM") as pss,
            tc.tile_pool(name="psa", bufs=1, space="PSUM") as psa,
            tc.tile_pool(name="pso", bufs=3, space="PSUM") as pso,
        ):
            qt_first = qtp.tile([128, nct, TC], F32, tag="qt")
            nc.sync.dma_start(
                out=qt_first,
                in_=qt_d.ap()[0, :, 0:TC].rearrange("(ct p) t -> p ct t", p=128))
            kq_sb = consts.tile([128, B_loc, nct, nW], F32)
            nc.sync.dma_start(
                out=kq_sb,
                in_=kq_d.ap().rearrange("b (ct p) w -> p b ct w", p=128))
            ident = consts.tile([128, 128], F32)
            nc.sync.dma_start(out=ident, in_=id_d.ap())
            vw_sb = consts.tile([nW, B_loc, C], F32)
            nc.sync.dma_start(out=vw_sb, in_=vw_d.ap().rearrange("b w c -> w b c"))
            vw_r = consts.tile([nW, B_loc, C], F32R)
            nc.vector.tensor_copy(vw_r, vw_sb)
            if use_sbias:
                sb_sb = consts.tile([128, B_loc, nW], F32)
                nc.gpsimd.dma_start(
                    out=sb_sb,
                    in_=bass.AP(tensor=sb_d.ap().tensor, offset=0,
                                ap=[[0, 128], [nW, B_loc], [1, nW]]))

            # warm the PE clock-gate (HAM) during the initial DMA wait;
            # memset source so the warmup depends on no DMA
            warm_src = consts.tile([128, 128], F32)
            nc.vector.memset(warm_src, 1.0)
            warm_ps = pst.tile([128, 128], F32, tag="scT")
            for _ in range(16):
                nc.tensor.transpose(warm_ps, warm_src, warm_src)

            for b in range(B_loc):
                attn_bt = smx.tile([128, n_chunks, n_sub, nW], F32, tag="attn")
                for ch in range(n_chunks):
                    t0 = ch * TC

                    # qT loaded directly (host supplies the transposed copy)
                    if b == 0 and ch == 0:
                        qt_sb = qt_first
                    else:
                        qt_sb = qtp.tile([128, nct, TC], F32, tag="qt")
                        nc.sync.dma_start(
                            out=qt_sb,
                            in_=qt_d.ap()[b, :, t0:t0 + TC]
                                .rearrange("(ct p) t -> p ct t", p=128))

                    # scores_T[w, t] = kq[b].T @ qT  (kq stationary: tiny LDW)
                    scT_ps = pst.tile([nW, TC], F32, tag="scT")
                    for ct in range(nct):
                        nc.tensor.matmul(
                            scT_ps, kq_sb[:, b, ct, :], qt_sb[:, ct, :],
                            start=(ct == 0), stop=(ct == nct - 1))
                    scT_sb = sctp.tile([nW, TC], F32, tag="scT_sb")
                    nc.any.tensor_copy(scT_sb, scT_ps)

                    # transpose back to [t, w] for the row-parallel softmax
                    sc_ps = pss.tile([128, n_sub, nW], F32, tag="sc_ps")
                    for s in range(n_sub):
                        nc.tensor.transpose(
                            sc_ps[:, s, :],
                            scT_sb[:, s * 128:(s + 1) * 128],
                            ident[0:nW, 0:nW])
                    if use_sbias:
                        for s in range(n_sub):
                            nc.vector.tensor_tensor(
                                sc_ps[:, s, :], sc_ps[:, s, :], sb_sb[:, b, :],
                                op=mybir.AluOpType.add)

                    # softmax over the nW window columns (batched over n_sub;
                    # max-subtraction skipped when host proves exp can't
                    # overflow for this data)
                    if nomax:
                        ex = smx.tile([128, n_sub, nW], F32, tag="ex")
                        nc.scalar.activation(ex, sc_ps,
                                             mybir.ActivationFunctionType.Exp)
                    else:
                        negmax = smx.tile([128, n_sub], F32, tag="negmax")
                        nc.vector.tensor_reduce(
                            negmax, sc_ps, axis=mybir.AxisListType.X,
                            op=mybir.AluOpType.max, negate=True)
                        negmax_bc = negmax[:, :, None].broadcast_to([128, n_sub, nW])
                        sh = smx.tile([128, n_sub, nW], F32, tag="sh")
                        nc.vector.tensor_tensor(sh, sc_ps, negmax_bc,
                                                op=mybir.AluOpType.add)
                        ex = smx.tile([128, n_sub, nW], F32, tag="ex")
                        nc.scalar.activation(ex, sh,
                                             mybir.ActivationFunctionType.Exp)
                    denom = smx.tile([128, n_sub], F32, tag="denom")
                    nc.vector.tensor_reduce(
                        denom, ex, axis=mybir.AxisListType.X,
                        op=mybir.AluOpType.add)
                    recip = smx.tile([128, n_sub], F32, tag="recip")
                    nc.vector.reciprocal(recip, denom)
                    attn_sb = attn_bt[:, ch, :, :]
                    nc.vector.tensor_tensor(attn_sb, ex,
                                            recip[:, :, None].broadcast_to([128, n_sub, nW]),
                                            op=mybir.AluOpType.mult)
                    if ch == n_chunks - 1:
                        nc.scalar.dma_start(
                            out=attn_d.ap()[b, :, :]
                                .rearrange("(c s p) w -> p c s w", p=128, s=n_sub),
                            in_=attn_bt)

                    # attn transposed to [w, t] for the output contraction
                    at_ps = psa.tile([nW, TC], F32, tag="at_ps")
                    for s in range(n_sub):
                        nc.tensor.transpose(
                            at_ps[:, s * 128:(s + 1) * 128],
                            attn_sb[:, s, :], ident)
                    at_r = atp.tile([nW, TC], F32R, tag="at_r")
                    nc.any.tensor_copy(at_r, at_ps)

                    # out2[t, c] = attn @ vw[b]  (residual added host-side)
                    fin = finp.tile([128, n_sub, C], F32, tag="fin")
                    for pair in range(n_sub // 2):
                        o2 = pso.tile([128, 2, C], F32, tag="o2")
                        for j in range(2):
                            s = pair * 2 + j
                            nc.tensor.matmul(
                                o2[:, j, :],
                                at_r[:, s * 128:(s + 1) * 128],
                                vw_r[:, b, :], start=True, stop=True)
                        nc.any.tensor_copy(fin[:, pair * 2:pair * 2 + 2, :], o2)
                    nc.scalar.dma_start(
                        out=out_d.ap()[b, t0:t0 + TC, :]
                            .rearrange("(s p) c -> p s c", p=128),
                        in_=fin)
    nc.compile()
    return nc


def kernel(query, keys, values, mask, W_in, b_in, W_out, b_out,
           last_attended, window_size):
    query = np.asarray(query, dtype=np.float32)
    keys = np.asarray(keys, dtype=np.float32)
    values = np.asarray(values, dtype=np.float32)
    mask = np.asarray(mask)
    W_in = np.asarray(W_in, dtype=np.float32)
    b_in = np.asarray(b_in, dtype=np.float32)
    W_out = np.asarray(W_out, dtype=np.float32)
    b_out = np.asarray(b_out, dtype=np.float32)
    la = int(last_attended)
    win = int(window_size)

    B, Tq, C = query.shape
    _, E, Tk = keys.shape
    lo, hi = max(0, la), min(Tk, la + win)
    assert hi > lo, "window has no valid column"
    nW = hi - lo
    assert B % N_CORES == 0
    B_loc = B // N_CORES

    # ---- host-side folding (float64) ----
    k_win = keys[:, :, lo:hi].astype(np.float64)          # [B, E, nW]
    v_win = values[:, lo:hi, :].astype(np.float64)        # [B, nW, E]
    mask_win = mask[:, lo:hi].astype(bool)                # [B, nW]
    out_scale = Tk * math.sqrt(1.0 / Tk)                  # = sqrt(Tk)

    # scores = (q @ W_in.T + b_in) @ k_win ; q fed pre-scaled by sqrt(.5)
    kq = np.einsum("ec,bew->bcw", W_in.astype(np.float64), k_win) / SQ05
    kq = np.ascontiguousarray(kq, dtype=np.float32)       # [B, C, nW]
    sbias = np.einsum("e,bew->bw", b_in.astype(np.float64), k_win)
    sbias = np.where(mask_win, -1e30, sbias).astype(np.float32)
    use_sbias = bool(np.any(sbias != 0.0))

    vw = np.einsum("bwe,ce->bwc", v_win, W_out.astype(np.float64))
    vw = np.ascontiguousarray(vw * (out_scale * SQ05), dtype=np.float32)
    qs = (query * np.float32(SQ05)).astype(np.float32)
    qsT = np.ascontiguousarray(qs.transpose(0, 2, 1))

    ident_np = np.eye(128, dtype=np.float32)
    key = (B_loc, Tq, C, nW, use_sbias, False)
    if key not in _cache:
        _cache[key] = _build(*key)
    nc = _cache[key]

    in_maps = []
    for c in range(N_CORES):
        s = slice(c * B_loc, (c + 1) * B_loc)
        m = {"qsT": qsT[s], "kq": kq[s], "vw": vw[s], "ident": ident_np}
        if use_sbias:
            m["sbias"] = sbias[s]
        in_maps.append(m)

    kw = {}
    if os.environ.get("BASS_KERNEL_TRACE"):
        try:
            from antenv.axon_hooks import get_axon_ntff_profile_hook  # noqa: F401
            import tempfile
            base = os.environ.get("BASS_KERNEL_TRACE_DIR") or tempfile.gettempdir()
            os.makedirs(base, exist_ok=True)
            kw = {"trace": True, "tmpdir": tempfile.mkdtemp(dir=base)}
        except ImportError:
            pass
    res = run_bass_kernel_spmd(nc, in_maps, list(range(N_CORES)), **kw)
    kernel.last_results = res

    out = np.concatenate([r["out"] for r in res.results], axis=0)
    out += qs
    if np.any(b_out != 0.0):
        out = out + (b_out * SQ05).astype(np.float32)
    attn = np.zeros((B, Tq, Tk), dtype=np.float32)
    attn[:, :, lo:hi] = np.concatenate([r["attnw"] for r in res.results], axis=0)
    return out, attn
